# revision 22
# baseline (speedup 1.0000x reference)
"""Multi-head causal attention (B=8, T=2048, C=384, H=6, Dh=64) on 8 TRN2 cores.

Sharding: data-parallel over batch — core b computes batch element b end to end
(no collectives).

v2 design notes (vs v1):
  - Head-PAIR processing: heads (2bi, 2bi+1) live on partition halves
    [0:64) / [64:128) of hd-block bi. Their S matmuls (K=64) are issued
    back-to-back so the PE runs them CONCURRENTLY via row tiling
    (tile_position auto-derived from base_partition 0 / 64).
  - One wide ACTIVATE (exp) per chunk-pair over S2 [128, 2, 512-d]
    (both heads at once) — halves ACT instruction overhead.
  - PV fringe-trimmed: matmul only cols [d:512] (P[:, :d] is never read
    → no memsets). Softmax denominator via V_aug ones-row (M=65).
  - Normalization: denominator rows broadcast across partitions on
    GpSimd (partition_broadcast), reciprocal on DVE, then one
    tensor_tensor multiply straight out of PSUM O → attT (bf16).
    No PE transposes, no per-head STT chains.
  - Output projection packed at K=128 (3 matmuls per 128-token block),
    bias added by the PSUM→SBUF scalar_tensor_tensor copy.
  - QKV projections are NOT a separate phase: proj for q-block j is
    emitted right before attention j, so the (readiness+priority)
    Tile scheduler uses proj matmuls as PE filler inside the
    ACT-limited attention stream — keeps the PE HAM-warm.
"""

import numpy as np
import ml_dtypes

import concourse.bass as bass
import concourse.tile as tile
from concourse import bacc, mybir
from concourse.bass import ts, ds

F32 = mybir.dt.float32
BF16 = mybir.dt.bfloat16
AF = mybir.ActivationFunctionType
ALU = mybir.AluOpType

B, T, C = 8, 2048, 384
H, DH = 6, 64
SCALE = DH ** -0.5
NCORES = 8
TJ = 512            # q-block width
NJ = T // TJ        # 4 q-blocks
SC = 128            # s-chunk
NCI = C // 128      # 3 channel chunks
NCH = TJ // SC      # fringe chunks per q-block (4)


def build_kernel():
    nc = bacc.Bacc("TRN2", target_bir_lowering=False, debug=False)

    xT_d = nc.dram_tensor("xT", [128, NCI, T], BF16, kind="ExternalInput").ap()
    wq_d = nc.dram_tensor("wq", [128, NCI, C], BF16, kind="ExternalInput").ap()
    wk_d = nc.dram_tensor("wk", [128, NCI, C], BF16, kind="ExternalInput").ap()
    wv_d = nc.dram_tensor("wv", [128, NCI, C], BF16, kind="ExternalInput").ap()
    wp_d = nc.dram_tensor("wp", [128, NCI, C], BF16, kind="ExternalInput").ap()
    biasb_d = nc.dram_tensor("biasb", [128, 384], F32, kind="ExternalInput").ap()
    y_d = nc.dram_tensor("y", [T, C], F32, kind="ExternalOutput").ap()

    with tile.TileContext(nc) as tc:
        with tc.tile_pool(name="const", bufs=1) as cpool, \
             tc.tile_pool(name="s2p", bufs=2, space="PSUM") as s2p, \
             tc.tile_pool(name="projp", bufs=1, space="PSUM") as projp, \
             tc.tile_pool(name="op", bufs=3, space="PSUM") as op, \
             tc.tile_pool(name="p2p", bufs=8) as p2p, \
             tc.tile_pool(name="r2p", bufs=3) as r2p, \
             tc.tile_pool(name="ysbp", bufs=3) as ysbp:
            xT = cpool.tile([128, NCI, T], BF16)
            wq = cpool.tile([128, NCI, C], BF16)
            wk = cpool.tile([128, NCI, C], BF16)
            wv = cpool.tile([128, NCI, C], BF16)
            wp = cpool.tile([128, NCI, C], BF16)
            biasb = cpool.tile([128, 384], F32)
            QT = cpool.tile([128, NCI, T], BF16)
            KT = cpool.tile([128, NCI, T], BF16)
            attT = cpool.tile([128, NCI, T], BF16)
            Vt = cpool.tile([128, 16, H, 65], BF16)

            nc.sync.dma_start(wq[:], wq_d[:])
            nc.sync.dma_start(wk[:], wk_d[:])
            nc.sync.dma_start(wv[:], wv_d[:])
            nc.sync.dma_start(wp[:], wp_d[:])
            nc.sync.dma_start(biasb[:], biasb_d[:])
            for tcn in range(NJ):
                for ci in range(NCI):
                    nc.sync.dma_start(xT[:, ci, ts(tcn, TJ)],
                                      xT_d[:, ci, ts(tcn, TJ)])
            # whole-tile memset (contiguous); V copies below overwrite
            # cols 0:64 of each [h, 65] group, leaving col 64 == 1.0
            nc.gpsimd.memset(Vt[:], 1.0)

            def proj_trios(j, spread=False):
                # projection work for q-block j as a list of closures,
                # so trios can be interleaved between attention pairs
                out = []
                for dst, w in ((KT, wk), (QT, wq)):
                    for pi in range(NCI):
                        def qk(dst=dst, w=w, pi=pi, pool_op=(spread and pi % 2)):
                            pool = op if pool_op else projp
                            tag = "O" if pool_op else "proj"
                            ps = pool.tile([128, TJ], F32, tag=tag, name="ps")
                            for ci in range(NCI):
                                nc.tensor.matmul(
                                    ps[:],
                                    lhsT=w[:, ci, ts(pi, 128)],
                                    rhs=xT[:, ci, ts(j, TJ)],
                                    start=(ci == 0), stop=(ci == NCI - 1),
                                )
                            nc.vector.tensor_copy(dst[:, pi, ts(j, TJ)], ps[:])
                        out.append(qk)
                for si in range(NCH * j, NCH * j + NCH):
                    def vp(si=si):
                        ps = projp.tile([128, TJ], F32, tag="proj", name="psv")
                        for ci in range(NCI):
                            nc.tensor.matmul(
                                ps[:, 0:C],
                                lhsT=xT[:, ci, ts(si, 128)],
                                rhs=wv[:, ci, :],
                                start=(ci == 0), stop=(ci == NCI - 1),
                            )
                        nc.vector.tensor_copy(
                            Vt[:, si, :, 0:64],
                            ps[:, 0:C].rearrange("p (h d) -> p h d", h=H),
                        )
                    out.append(vp)
                return out

            def emit_outproj(j, spread=False):
                # output projection for q-block j
                for q in range(NCH):
                    tb = NCH * j + q
                    if spread and q % 2:
                        Y = op.tile([128, TJ], F32, tag="O", name="Y")[:, 0:C]
                    else:
                        Y = projp.tile([128, TJ], F32, tag="proj", name="Y")[:, 0:C]
                    for bi in range(NCI):
                        nc.tensor.matmul(
                            Y[:],
                            lhsT=attT[:, bi, ts(tb, 128)],
                            rhs=wp[:, bi, :],
                            start=(bi == 0), stop=(bi == NCI - 1),
                        )
                    Ysb = ysbp.tile([128, C], F32, tag="Ysb")
                    nc.vector.scalar_tensor_tensor(
                        out=Ysb[:], in0=Y[:], scalar=1.0, in1=biasb[:],
                        op0=ALU.mult, op1=ALU.add,
                    )
                    nc.sync.dma_start(y_d[ts(tb, 128), :], Ysb[:])

            # j0 proj upfront, spread across both PSUM pools for ramp speed
            for t in proj_trios(0, spread=True):
                t()
            for j in range(NJ):
                # next block's proj trios get interleaved between this
                # block's attention pairs (PE filler in the static order)
                pending = proj_trios(j + 1) if j + 1 < NJ else []

                # ---- attention for q-block j, head pairs ----
                nch = NCH * j + NCH  # s-chunks for this q-block
                for bi in range(NCI):
                    h0, h1 = 2 * bi, 2 * bi + 1
                    O0 = op.tile([65, TJ], F32, tag="O")
                    O1 = op.tile([65, TJ], F32, tag="O")
                    for i in range(nch):
                        fringe = i >= NCH * j
                        d = SC * i - TJ * j if fringe else 0
                        S2 = s2p.tile([128, 2, TJ], F32, tag="S2")
                        # the two heads' S matmuls target different PE row
                        # groups (K=64 at partitions 0/64) → run concurrently
                        nc.tensor.matmul(
                            S2[:, 0, d:TJ],
                            lhsT=KT[0:64, bi, ts(i, SC)],
                            rhs=QT[0:64, bi, ds(j * TJ + d, TJ - d)],
                            start=True, stop=True,
                        )
                        nc.tensor.matmul(
                            S2[:, 1, d:TJ],
                            lhsT=KT[64:128, bi, ts(i, SC)],
                            rhs=QT[64:128, bi, ds(j * TJ + d, TJ - d)],
                            start=True, stop=True,
                        )
                        P2 = p2p.tile([128, 2, TJ], BF16, tag="P2")
                        nc.scalar.activation(P2[:, :, d:TJ], S2[:, :, d:TJ],
                                             AF.Exp, scale=SCALE)
                        if fringe:
                            # diagonal window [d, d+128): keep iff p <= f
                            for half in range(2):
                                nc.gpsimd.affine_select(
                                    out=P2[:, half, d:d + SC],
                                    in_=P2[:, half, d:d + SC],
                                    pattern=[[1, SC]],
                                    compare_op=ALU.is_ge,
                                    fill=0.0, base=0, channel_multiplier=-1,
                                )
                        nc.tensor.matmul(
                            O0[:, d:TJ],
                            lhsT=Vt[:, i, h0, :],
                            rhs=P2[:, 0, d:TJ],
                            start=(i == 0), stop=(i == nch - 1),
                        )
                        nc.tensor.matmul(
                            O1[:, d:TJ],
                            lhsT=Vt[:, i, h1, :],
                            rhs=P2[:, 1, d:TJ],
                            start=(i == 0), stop=(i == nch - 1),
                        )
    # normalization: reciprocal of the denom rows straight out of
                    # PSUM (approx_fast: ~18 bits, denominators are >= 1),
                    # broadcast over partitions 0:64 on gpsimd, multiply.
                    # All DVE input operands stay at partition base 0
                    # (mismatched in0/in1 bases read wrong data); only
                    # outputs are partition-shifted.
                    dA = r2p.tile([1, TJ], F32, tag="dA")
                    dB = r2p.tile([1, TJ], F32, tag="dB")
                    rA = r2p.tile([1, TJ], F32, tag="rA")
                    rB = r2p.tile([1, TJ], F32, tag="rB")
                    nc.vector.tensor_copy(dA[:], O0[64:65, :])
                    nc.vector.tensor_copy(dB[:], O1[64:65, :])
                    nc.vector.reciprocal_approx_fast(rA[:], dA[:])
                    nc.vector.reciprocal_approx_fast(rB[:], dB[:])
                    RA = r2p.tile([64, TJ], F32, tag="RA")
                    RB = r2p.tile([64, TJ], F32, tag="RB")
                    nc.gpsimd.partition_broadcast(RA[:], rA[:])
                    nc.gpsimd.partition_broadcast(RB[:], rB[:])
                    nc.vector.tensor_tensor(
                        out=attT[0:64, bi, ts(j, TJ)], in0=O0[0:64, :],
                        in1=RA[:], op=ALU.mult,
                    )
                    nc.vector.tensor_tensor(
                        out=attT[64:128, bi, ts(j, TJ)], in0=O1[0:64, :],
                        in1=RB[:], op=ALU.mult,
                    )
                    # interleave next-block proj trios between pairs,
                    # back-loaded to cover the j-boundary pipeline drain
                    nshare = (2, 2, 10)[bi] if pending else 0
                    for _ in range(min(nshare, len(pending))):
                        pending.pop(0)()

                emit_outproj(j, spread=(j == NJ - 1))

    nc.compile()
    return nc


def _prep_inputs(x, Wq, Wk, Wv, Wp, bp):
    """Host-side shard + layout prep. Returns per-core input maps."""
    bf = ml_dtypes.bfloat16
    x = np.asarray(x, dtype=np.float32)

    def pack_w(W):  # [H, C, Dh] -> [128, NCI, H*Dh]
        Whd = np.transpose(np.asarray(W, np.float32), (1, 0, 2)).reshape(C, H * DH)
        return np.ascontiguousarray(
            Whd.reshape(NCI, 128, H * DH).transpose(1, 0, 2)
        ).astype(bf)

    wq_p, wk_p, wv_p = pack_w(Wq), pack_w(Wk), pack_w(Wv)
    wp_p = np.ascontiguousarray(
        np.asarray(Wp, np.float32).reshape(NCI, 128, C).transpose(1, 0, 2)
    ).astype(bf)

    biasb = np.broadcast_to(np.asarray(bp, np.float32), (128, C)).copy()

    in_maps = []
    for b in range(B):
        xT = np.ascontiguousarray(
            x[b].T.reshape(NCI, 128, T).transpose(1, 0, 2)
        ).astype(bf)
        in_maps.append({
            "xT": xT, "wq": wq_p, "wk": wk_p, "wv": wv_p, "wp": wp_p,
            "biasb": biasb,
        })
    return in_maps


_CACHE = {}


def kernel(x, Wq, Wk, Wv, Wp, bp):
    from concourse.bass_utils import run_bass_kernel_spmd

    if "nc" not in _CACHE:
        _CACHE["nc"] = build_kernel()
    nc = _CACHE["nc"]
    in_maps = _prep_inputs(x, Wq, Wk, Wv, Wp, bp)
    res = run_bass_kernel_spmd(nc, in_maps, list(range(NCORES)))
    out = np.stack([res.results[b]["y"] for b in range(B)], axis=0)
    return out.astype(np.float32)


# revision 23
# speedup vs baseline: 1.0056x; 1.0056x over previous
"""Multi-head causal attention (B=8, T=2048, C=384, H=6, Dh=64) on 8 TRN2 cores.

Sharding: data-parallel over batch — core b computes batch element b end to end
(no collectives).

v2 design notes (vs v1):
  - Head-PAIR processing: heads (2bi, 2bi+1) live on partition halves
    [0:64) / [64:128) of hd-block bi. Their S matmuls (K=64) are issued
    back-to-back so the PE runs them CONCURRENTLY via row tiling
    (tile_position auto-derived from base_partition 0 / 64).
  - One wide ACTIVATE (exp) per chunk-pair over S2 [128, 2, 512-d]
    (both heads at once) — halves ACT instruction overhead.
  - PV fringe-trimmed: matmul only cols [d:512] (P[:, :d] is never read
    → no memsets). Softmax denominator via V_aug ones-row (M=65).
  - Normalization: denominator rows broadcast across partitions on
    GpSimd (partition_broadcast), reciprocal on DVE, then one
    tensor_tensor multiply straight out of PSUM O → attT (bf16).
    No PE transposes, no per-head STT chains.
  - Output projection packed at K=128 (3 matmuls per 128-token block),
    bias added by the PSUM→SBUF scalar_tensor_tensor copy.
  - QKV projections are NOT a separate phase: block j+1's projection
    trios are emitted interleaved between block j's attention pairs, so
    the (readiness+priority) Tile scheduler uses them as PE filler
    inside the latency-bound attention chunk chain — keeps the PE
    HAM-warm (cold-clock dropped ~128us -> ~32us).
  - PSUM budget (8 banks): S2 pair tiles 2x2, proj/Y 1, O pairs 3.
    Pool membership chosen so no attention-critical alloc waits on the
    outproj/proj rotation (Y lives in the proj pool, not the O pool).
"""

import numpy as np
import ml_dtypes

import concourse.bass as bass
import concourse.tile as tile
from concourse import bacc, mybir
from concourse.bass import ts, ds

F32 = mybir.dt.float32
BF16 = mybir.dt.bfloat16
AF = mybir.ActivationFunctionType
ALU = mybir.AluOpType

B, T, C = 8, 2048, 384
H, DH = 6, 64
SCALE = DH ** -0.5
NCORES = 8
TJ = 512            # q-block width
NJ = T // TJ        # 4 q-blocks
SC = 128            # s-chunk
NCI = C // 128      # 3 channel chunks
NCH = TJ // SC      # fringe chunks per q-block (4)


def build_kernel():
    nc = bacc.Bacc("TRN2", target_bir_lowering=False, debug=False)

    xT_d = nc.dram_tensor("xT", [128, NCI, T], BF16, kind="ExternalInput").ap()
    wq_d = nc.dram_tensor("wq", [128, NCI, C], BF16, kind="ExternalInput").ap()
    wk_d = nc.dram_tensor("wk", [128, NCI, C], BF16, kind="ExternalInput").ap()
    wv_d = nc.dram_tensor("wv", [128, NCI, C], BF16, kind="ExternalInput").ap()
    wp_d = nc.dram_tensor("wp", [128, NCI, C], BF16, kind="ExternalInput").ap()
    biasb_d = nc.dram_tensor("biasb", [128, 384], F32, kind="ExternalInput").ap()
    y_d = nc.dram_tensor("y", [T, C], F32, kind="ExternalOutput").ap()

    with tile.TileContext(nc) as tc:
        with tc.tile_pool(name="const", bufs=1) as cpool, \
             tc.tile_pool(name="s2p", bufs=2, space="PSUM") as s2p, \
             tc.tile_pool(name="projp", bufs=1, space="PSUM") as projp, \
             tc.tile_pool(name="op", bufs=3, space="PSUM") as op, \
             tc.tile_pool(name="p2p", bufs=8) as p2p, \
             tc.tile_pool(name="r2p", bufs=3) as r2p, \
             tc.tile_pool(name="ysbp", bufs=3) as ysbp:
            xT = cpool.tile([128, NCI, T], BF16)
            wq = cpool.tile([128, NCI, C], BF16)
            wk = cpool.tile([128, NCI, C], BF16)
            wv = cpool.tile([128, NCI, C], BF16)
            wp = cpool.tile([128, NCI, C], BF16)
            biasb = cpool.tile([128, 384], F32)
            QT = cpool.tile([128, NCI, T], BF16)
            KT = cpool.tile([128, NCI, T], BF16)
            attT = cpool.tile([128, NCI, T], BF16)
            Vt = cpool.tile([128, 16, H, 65], BF16)

            nc.sync.dma_start(wq[:], wq_d[:])
            nc.sync.dma_start(wk[:], wk_d[:])
            nc.sync.dma_start(wv[:], wv_d[:])
            nc.sync.dma_start(wp[:], wp_d[:])
            nc.sync.dma_start(biasb[:], biasb_d[:])
            for tcn in range(NJ):
                for ci in range(NCI):
                    nc.sync.dma_start(xT[:, ci, ts(tcn, TJ)],
                                      xT_d[:, ci, ts(tcn, TJ)])
            # whole-tile memset (contiguous); V copies below overwrite
            # cols 0:64 of each [h, 65] group, leaving col 64 == 1.0
            nc.gpsimd.memset(Vt[:], 1.0)

            def proj_trios(j, spread=False):
                # projection work for q-block j as a list of closures,
                # so trios can be interleaved between attention pairs
                out = []
                for dst, w in ((KT, wk), (QT, wq)):
                    for pi in range(NCI):
                        def qk(dst=dst, w=w, pi=pi, pool_op=(spread and pi % 2)):
                            pool = op if pool_op else projp
                            tag = "O" if pool_op else "proj"
                            ps = pool.tile([128, TJ], F32, tag=tag, name="ps")
                            for ci in range(NCI):
                                nc.tensor.matmul(
                                    ps[:],
                                    lhsT=w[:, ci, ts(pi, 128)],
                                    rhs=xT[:, ci, ts(j, TJ)],
                                    start=(ci == 0), stop=(ci == NCI - 1),
                                )
                            nc.vector.tensor_copy(dst[:, pi, ts(j, TJ)], ps[:])
                        out.append(qk)
                for si in range(NCH * j, NCH * j + NCH):
                    def vp(si=si):
                        ps = projp.tile([128, TJ], F32, tag="proj", name="psv")
                        for ci in range(NCI):
                            nc.tensor.matmul(
                                ps[:, 0:C],
                                lhsT=xT[:, ci, ts(si, 128)],
                                rhs=wv[:, ci, :],
                                start=(ci == 0), stop=(ci == NCI - 1),
                            )
                        nc.vector.tensor_copy(
                            Vt[:, si, :, 0:64],
                            ps[:, 0:C].rearrange("p (h d) -> p h d", h=H),
                        )
                    out.append(vp)
                return out

            def emit_outproj(j, spread=False):
                # output projection for q-block j
                for q in range(NCH):
                    tb = NCH * j + q
                    if spread and q % 2:
                        Y = op.tile([128, TJ], F32, tag="O", name="Y")[:, 0:C]
                    else:
                        Y = projp.tile([128, TJ], F32, tag="proj", name="Y")[:, 0:C]
                    for bi in range(NCI):
                        nc.tensor.matmul(
                            Y[:],
                            lhsT=attT[:, bi, ts(tb, 128)],
                            rhs=wp[:, bi, :],
                            start=(bi == 0), stop=(bi == NCI - 1),
                        )
                    Ysb = ysbp.tile([128, C], F32, tag="Ysb")
                    nc.vector.scalar_tensor_tensor(
                        out=Ysb[:], in0=Y[:], scalar=1.0, in1=biasb[:],
                        op0=ALU.mult, op1=ALU.add,
                    )
                    nc.sync.dma_start(y_d[ts(tb, 128), :], Ysb[:])

            # j0 proj upfront, spread across both PSUM pools for ramp speed
            for t in proj_trios(0, spread=True):
                t()
            for j in range(NJ):
                # next block's proj trios get interleaved between this
                # block's attention pairs (PE filler in the static order)
                pending = proj_trios(j + 1) if j + 1 < NJ else []

                # ---- attention for q-block j, head pairs ----
                nch = NCH * j + NCH  # s-chunks for this q-block
                for bi in range(NCI):
                    h0, h1 = 2 * bi, 2 * bi + 1
                    O0 = op.tile([65, TJ], F32, tag="O")
                    O1 = op.tile([65, TJ], F32, tag="O")
                    for i in range(nch):
                        fringe = i >= NCH * j
                        d = SC * i - TJ * j if fringe else 0
                        S2 = s2p.tile([128, 2, TJ], F32, tag="S2")
                        # the two heads' S matmuls target different PE row
                        # groups (K=64 at partitions 0/64) → run concurrently
                        nc.tensor.matmul(
                            S2[:, 0, d:TJ],
                            lhsT=KT[0:64, bi, ts(i, SC)],
                            rhs=QT[0:64, bi, ds(j * TJ + d, TJ - d)],
                            start=True, stop=True,
                        )
                        nc.tensor.matmul(
                            S2[:, 1, d:TJ],
                            lhsT=KT[64:128, bi, ts(i, SC)],
                            rhs=QT[64:128, bi, ds(j * TJ + d, TJ - d)],
                            start=True, stop=True,
                        )
                        P2 = p2p.tile([128, 2, TJ], BF16, tag="P2")
                        nc.scalar.activation(P2[:, :, d:TJ], S2[:, :, d:TJ],
                                             AF.Exp, scale=SCALE)
                        if fringe:
                            # diagonal window [d, d+128): keep iff p <= f
                            for half in range(2):
                                nc.gpsimd.affine_select(
                                    out=P2[:, half, d:d + SC],
                                    in_=P2[:, half, d:d + SC],
                                    pattern=[[1, SC]],
                                    compare_op=ALU.is_ge,
                                    fill=0.0, base=0, channel_multiplier=-1,
                                )
                        nc.tensor.matmul(
                            O0[:, d:TJ],
                            lhsT=Vt[:, i, h0, :],
                            rhs=P2[:, 0, d:TJ],
                            start=(i == 0), stop=(i == nch - 1),
                        )
                        nc.tensor.matmul(
                            O1[:, d:TJ],
                            lhsT=Vt[:, i, h1, :],
                            rhs=P2[:, 1, d:TJ],
                            start=(i == 0), stop=(i == nch - 1),
                        )
    # normalization: reciprocal of the denom rows straight out of
                    # PSUM (approx_fast: ~18 bits, denominators are >= 1),
                    # broadcast over partitions 0:64 on gpsimd, multiply.
                    # All DVE input operands stay at partition base 0
                    # (mismatched in0/in1 bases read wrong data); only
                    # outputs are partition-shifted.
                    dA = r2p.tile([1, TJ], F32, tag="dA")
                    dB = r2p.tile([1, TJ], F32, tag="dB")
                    rA = r2p.tile([1, TJ], F32, tag="rA")
                    rB = r2p.tile([1, TJ], F32, tag="rB")
                    nc.vector.tensor_copy(dA[:], O0[64:65, :])
                    nc.vector.tensor_copy(dB[:], O1[64:65, :])
                    nc.vector.reciprocal_approx_fast(rA[:], dA[:])
                    nc.vector.reciprocal_approx_fast(rB[:], dB[:])
                    RA = r2p.tile([64, TJ], F32, tag="RA")
                    RB = r2p.tile([64, TJ], F32, tag="RB")
                    nc.gpsimd.partition_broadcast(RA[:], rA[:])
                    nc.gpsimd.partition_broadcast(RB[:], rB[:])
                    nc.vector.tensor_tensor(
                        out=attT[0:64, bi, ts(j, TJ)], in0=O0[0:64, :],
                        in1=RA[:], op=ALU.mult,
                    )
                    nc.vector.tensor_tensor(
                        out=attT[64:128, bi, ts(j, TJ)], in0=O1[0:64, :],
                        in1=RB[:], op=ALU.mult,
                    )
                    # interleave next-block proj trios between pairs,
                    # back-loaded to cover the j-boundary pipeline drain
                    nshare = (2, 2, 10)[bi] if pending else 0
                    for _ in range(min(nshare, len(pending))):
                        pending.pop(0)()

                emit_outproj(j, spread=(j == NJ - 1))

    nc.compile()
    return nc


def _prep_inputs(x, Wq, Wk, Wv, Wp, bp):
    """Host-side shard + layout prep. Returns per-core input maps."""
    bf = ml_dtypes.bfloat16
    x = np.asarray(x, dtype=np.float32)

    def pack_w(W):  # [H, C, Dh] -> [128, NCI, H*Dh]
        Whd = np.transpose(np.asarray(W, np.float32), (1, 0, 2)).reshape(C, H * DH)
        return np.ascontiguousarray(
            Whd.reshape(NCI, 128, H * DH).transpose(1, 0, 2)
        ).astype(bf)

    wq_p, wk_p, wv_p = pack_w(Wq), pack_w(Wk), pack_w(Wv)
    wp_p = np.ascontiguousarray(
        np.asarray(Wp, np.float32).reshape(NCI, 128, C).transpose(1, 0, 2)
    ).astype(bf)

    biasb = np.broadcast_to(np.asarray(bp, np.float32), (128, C)).copy()

    in_maps = []
    for b in range(B):
        xT = np.ascontiguousarray(
            x[b].T.reshape(NCI, 128, T).transpose(1, 0, 2)
        ).astype(bf)
        in_maps.append({
            "xT": xT, "wq": wq_p, "wk": wk_p, "wv": wv_p, "wp": wp_p,
            "biasb": biasb,
        })
    return in_maps


_CACHE = {}


def kernel(x, Wq, Wk, Wv, Wp, bp):
    from concourse.bass_utils import run_bass_kernel_spmd

    if "nc" not in _CACHE:
        _CACHE["nc"] = build_kernel()
    nc = _CACHE["nc"]
    in_maps = _prep_inputs(x, Wq, Wk, Wv, Wp, bp)
    res = run_bass_kernel_spmd(nc, in_maps, list(range(NCORES)))
    out = np.stack([res.results[b]["y"] for b in range(B)], axis=0)
    return out.astype(np.float32)


# revision 26
# speedup vs baseline: 1.0196x; 1.0140x over previous
"""Multi-head causal attention (B=8, T=2048, C=384, H=6, Dh=64) on 8 TRN2 cores.

Sharding: data-parallel over batch — core b computes batch element b end to end
(no collectives).

v2 design notes (vs v1):
  - Head-PAIR processing: heads (2bi, 2bi+1) live on partition halves
    [0:64) / [64:128) of hd-block bi. Their S matmuls (K=64) are issued
    back-to-back so the PE runs them CONCURRENTLY via row tiling
    (tile_position auto-derived from base_partition 0 / 64).
  - One wide ACTIVATE (exp) per chunk-pair over S2 [128, 2, 512-d]
    (both heads at once) — halves ACT instruction overhead.
  - PV fringe-trimmed: matmul only cols [d:512] (P[:, :d] is never read
    → no memsets). Softmax denominator via V_aug ones-row (M=65).
  - Normalization: denominator rows broadcast across partitions on
    GpSimd (partition_broadcast), reciprocal on DVE, then one
    tensor_tensor multiply straight out of PSUM O → attT (bf16).
    No PE transposes, no per-head STT chains.
  - Output projection packed at K=128 (3 matmuls per 128-token block),
    bias added by the PSUM→SBUF scalar_tensor_tensor copy.
  - QKV projections are NOT a separate phase: block j+1's projection
    trios are emitted interleaved between block j's attention pairs, so
    the (readiness+priority) Tile scheduler uses them as PE filler
    inside the latency-bound attention chunk chain — keeps the PE
    HAM-warm (cold-clock dropped ~128us -> ~32us).
  - PSUM budget (8 banks): S2 pair tiles 2x2, proj/Y 1, O pairs 3.
    Pool membership chosen so no attention-critical alloc waits on the
    outproj/proj rotation (Y lives in the proj pool, not the O pool).
"""

import numpy as np
import ml_dtypes

import concourse.bass as bass
import concourse.tile as tile
from concourse import bacc, mybir
from concourse.bass import ts, ds

F32 = mybir.dt.float32
BF16 = mybir.dt.bfloat16
AF = mybir.ActivationFunctionType
ALU = mybir.AluOpType

B, T, C = 8, 2048, 384
H, DH = 6, 64
SCALE = DH ** -0.5
NCORES = 8
TJ = 512            # q-block width
NJ = T // TJ        # 4 q-blocks
SC = 128            # s-chunk
NCI = C // 128      # 3 channel chunks
NCH = TJ // SC      # fringe chunks per q-block (4)


def build_kernel():
    nc = bacc.Bacc("TRN2", target_bir_lowering=False, debug=False)

    xT_d = nc.dram_tensor("xT", [128, NCI, T], BF16, kind="ExternalInput").ap()
    wq_d = nc.dram_tensor("wq", [128, NCI, C], BF16, kind="ExternalInput").ap()
    wk_d = nc.dram_tensor("wk", [128, NCI, C], BF16, kind="ExternalInput").ap()
    wv_d = nc.dram_tensor("wv", [128, NCI, C], BF16, kind="ExternalInput").ap()
    wp_d = nc.dram_tensor("wp", [128, NCI, C], BF16, kind="ExternalInput").ap()
    biasb_d = nc.dram_tensor("biasb", [128, 384], F32, kind="ExternalInput").ap()
    y_d = nc.dram_tensor("y", [T, C], F32, kind="ExternalOutput").ap()

    with tile.TileContext(nc) as tc:
        with tc.tile_pool(name="const", bufs=1) as cpool, \
             tc.tile_pool(name="s2p", bufs=2, space="PSUM") as s2p, \
             tc.tile_pool(name="projp", bufs=1, space="PSUM") as projp, \
             tc.tile_pool(name="op", bufs=3, space="PSUM") as op, \
             tc.tile_pool(name="p2p", bufs=8) as p2p, \
             tc.tile_pool(name="r2p", bufs=3) as r2p, \
             tc.tile_pool(name="ysbp", bufs=3) as ysbp:
            xT = cpool.tile([128, NCI, T], BF16)
            wq = cpool.tile([128, NCI, C], BF16)
            wk = cpool.tile([128, NCI, C], BF16)
            wv = cpool.tile([128, NCI, C], BF16)
            wp = cpool.tile([128, NCI, C], BF16)
            biasb = cpool.tile([128, 384], F32)
            QT = cpool.tile([128, NCI, T], BF16)
            KT = cpool.tile([128, NCI, T], BF16)
            attT = cpool.tile([128, NCI, T], BF16)
            Vt = cpool.tile([128, 16, H, 65], BF16)

            nc.sync.dma_start(wq[:], wq_d[:])
            nc.sync.dma_start(wk[:], wk_d[:])
            nc.sync.dma_start(wv[:], wv_d[:])
            nc.sync.dma_start(wp[:], wp_d[:])
            nc.sync.dma_start(biasb[:], biasb_d[:])
            for tcn in range(NJ):
                for ci in range(NCI):
                    nc.sync.dma_start(xT[:, ci, ts(tcn, TJ)],
                                      xT_d[:, ci, ts(tcn, TJ)])
            # whole-tile memset (contiguous); V copies below overwrite
            # cols 0:64 of each [h, 65] group, leaving col 64 == 1.0
            nc.gpsimd.memset(Vt[:], 1.0)

            def qk_trio(dst, w, tcn, pi, pool_op=False):
                def run():
                    pool = op if pool_op else projp
                    tag = "O" if pool_op else "proj"
                    ps = pool.tile([128, TJ], F32, tag=tag, name="ps")
                    for ci in range(NCI):
                        nc.tensor.matmul(
                            ps[:],
                            lhsT=w[:, ci, ts(pi, 128)],
                            rhs=xT[:, ci, ts(tcn, TJ)],
                            start=(ci == 0), stop=(ci == NCI - 1),
                        )
                    nc.vector.tensor_copy(dst[:, pi, ts(tcn, TJ)], ps[:])
                return run

            def v_trio(si, pool_op=False):
                def run():
                    pool = op if pool_op else projp
                    tag = "O" if pool_op else "proj"
                    ps = pool.tile([128, TJ], F32, tag=tag, name="psv")
                    for ci in range(NCI):
                        nc.tensor.matmul(
                            ps[:, 0:C],
                            lhsT=xT[:, ci, ts(si, 128)],
                            rhs=wv[:, ci, :],
                            start=(ci == 0), stop=(ci == NCI - 1),
                        )
                    nc.vector.tensor_copy(
                        Vt[:, si, :, 0:64],
                        ps[:, 0:C].rearrange("p (h d) -> p h d", h=H),
                    )
                return run

            def emit_outproj(j, spread=False):
                # output projection for q-block j
                for q in range(NCH):
                    tb = NCH * j + q
                    if spread and q % 2:
                        Y = op.tile([128, TJ], F32, tag="O", name="Y")[:, 0:C]
                    else:
                        Y = projp.tile([128, TJ], F32, tag="proj", name="Y")[:, 0:C]
                    for bi in range(NCI):
                        nc.tensor.matmul(
                            Y[:],
                            lhsT=attT[:, bi, ts(tb, 128)],
                            rhs=wp[:, bi, :],
                            start=(bi == 0), stop=(bi == NCI - 1),
                        )
                    Ysb = ysbp.tile([128, C], F32, tag="Ysb")
                    nc.vector.scalar_tensor_tensor(
                        out=Ysb[:], in0=Y[:], scalar=1.0, in1=biasb[:],
                        op0=ALU.mult, op1=ALU.add,
                    )
                    nc.sync.dma_start(y_d[ts(tb, 128), :], Ysb[:])

            # Process q-blocks in order [1, 2, 3, 0]: long blocks first so
            # every j-boundary pipeline drain is absorbed by following
            # work, and tiny j0 covers j3's drain at the end.
            JORDER = [1, 2, 3, 0]
            # upfront: everything attention(j1) needs — KT cols 0:1024
            # (tcn 0+1), QT block 1, V chunks 0..7 — ordered so pair bi=0
            # becomes ready first; spread across both PSUM pools
            upfront = []
            for pi in range(NCI):
                upfront += [qk_trio(KT, wk, 0, pi, pool_op=False),
                            qk_trio(KT, wk, 1, pi, pool_op=True),
                            qk_trio(QT, wq, 1, pi, pool_op=False)]
            upfront += [v_trio(si, pool_op=(si % 2 == 1)) for si in range(8)]
            # filler trios for each later block
            TRIOS = {
                2: [qk_trio(KT, wk, 2, pi) for pi in range(NCI)]
                   + [qk_trio(QT, wq, 2, pi) for pi in range(NCI)]
                   + [v_trio(si) for si in range(8, 12)],
                3: [qk_trio(KT, wk, 3, pi) for pi in range(NCI)]
                   + [qk_trio(QT, wq, 3, pi) for pi in range(NCI)]
                   + [v_trio(si) for si in range(12, 16)],
                0: [qk_trio(QT, wq, 0, pi) for pi in range(NCI)],
            }
            for t in upfront:
                t()
            for jx, j in enumerate(JORDER):
                # next block's proj trios get interleaved between this
                # block's attention pairs (PE filler in the static order)
                pending = TRIOS.get(JORDER[jx + 1], []) \
                    if jx + 1 < len(JORDER) else []

                # ---- attention for q-block j, head pairs ----
                nch = NCH * j + NCH  # s-chunks for this q-block
                for bi in range(NCI):
                    h0, h1 = 2 * bi, 2 * bi + 1
                    O0 = op.tile([65, TJ], F32, tag="O")
                    O1 = op.tile([65, TJ], F32, tag="O")
                    for i in range(nch):
                        fringe = i >= NCH * j
                        d = SC * i - TJ * j if fringe else 0
                        S2 = s2p.tile([128, 2, TJ], F32, tag="S2")
                        # the two heads' S matmuls target different PE row
                        # groups (K=64 at partitions 0/64) → run concurrently
                        nc.tensor.matmul(
                            S2[:, 0, d:TJ],
                            lhsT=KT[0:64, bi, ts(i, SC)],
                            rhs=QT[0:64, bi, ds(j * TJ + d, TJ - d)],
                            start=True, stop=True,
                        )
                        nc.tensor.matmul(
                            S2[:, 1, d:TJ],
                            lhsT=KT[64:128, bi, ts(i, SC)],
                            rhs=QT[64:128, bi, ds(j * TJ + d, TJ - d)],
                            start=True, stop=True,
                        )
                        P2 = p2p.tile([128, 2, TJ], BF16, tag="P2")
                        nc.scalar.activation(P2[:, :, d:TJ], S2[:, :, d:TJ],
                                             AF.Exp, scale=SCALE)
                        if fringe:
                            # diagonal window [d, d+128): keep iff p <= f
                            for half in range(2):
                                nc.gpsimd.affine_select(
                                    out=P2[:, half, d:d + SC],
                                    in_=P2[:, half, d:d + SC],
                                    pattern=[[1, SC]],
                                    compare_op=ALU.is_ge,
                                    fill=0.0, base=0, channel_multiplier=-1,
                                )
                        nc.tensor.matmul(
                            O0[:, d:TJ],
                            lhsT=Vt[:, i, h0, :],
                            rhs=P2[:, 0, d:TJ],
                            start=(i == 0), stop=(i == nch - 1),
                        )
                        nc.tensor.matmul(
                            O1[:, d:TJ],
                            lhsT=Vt[:, i, h1, :],
                            rhs=P2[:, 1, d:TJ],
                            start=(i == 0), stop=(i == nch - 1),
                        )
    # normalization: reciprocal of the denom rows straight out of
                    # PSUM (approx_fast: ~18 bits, denominators are >= 1),
                    # broadcast over partitions 0:64 on gpsimd, multiply.
                    # All DVE input operands stay at partition base 0
                    # (mismatched in0/in1 bases read wrong data); only
                    # outputs are partition-shifted.
                    dA = r2p.tile([1, TJ], F32, tag="dA")
                    dB = r2p.tile([1, TJ], F32, tag="dB")
                    rA = r2p.tile([1, TJ], F32, tag="rA")
                    rB = r2p.tile([1, TJ], F32, tag="rB")
                    nc.vector.tensor_copy(dA[:], O0[64:65, :])
                    nc.vector.tensor_copy(dB[:], O1[64:65, :])
                    nc.vector.reciprocal_approx_fast(rA[:], dA[:])
                    nc.vector.reciprocal_approx_fast(rB[:], dB[:])
                    RA = r2p.tile([64, TJ], F32, tag="RA")
                    RB = r2p.tile([64, TJ], F32, tag="RB")
                    nc.gpsimd.partition_broadcast(RA[:], rA[:])
                    nc.gpsimd.partition_broadcast(RB[:], rB[:])
                    nc.vector.tensor_tensor(
                        out=attT[0:64, bi, ts(j, TJ)], in0=O0[0:64, :],
                        in1=RA[:], op=ALU.mult,
                    )
                    nc.vector.tensor_tensor(
                        out=attT[64:128, bi, ts(j, TJ)], in0=O1[0:64, :],
                        in1=RB[:], op=ALU.mult,
                    )
                    # interleave next-block proj trios between pairs,
                    # back-loaded to cover the j-boundary pipeline drain
                    nshare = (2, 2, 10)[bi] if pending else 0
                    for _ in range(min(nshare, len(pending))):
                        pending.pop(0)()

                emit_outproj(j, spread=(jx == len(JORDER) - 1))

    nc.compile()
    return nc


def _prep_inputs(x, Wq, Wk, Wv, Wp, bp):
    """Host-side shard + layout prep. Returns per-core input maps."""
    bf = ml_dtypes.bfloat16
    x = np.asarray(x, dtype=np.float32)

    def pack_w(W):  # [H, C, Dh] -> [128, NCI, H*Dh]
        Whd = np.transpose(np.asarray(W, np.float32), (1, 0, 2)).reshape(C, H * DH)
        return np.ascontiguousarray(
            Whd.reshape(NCI, 128, H * DH).transpose(1, 0, 2)
        ).astype(bf)

    wq_p, wk_p, wv_p = pack_w(Wq), pack_w(Wk), pack_w(Wv)
    wp_p = np.ascontiguousarray(
        np.asarray(Wp, np.float32).reshape(NCI, 128, C).transpose(1, 0, 2)
    ).astype(bf)

    biasb = np.broadcast_to(np.asarray(bp, np.float32), (128, C)).copy()

    in_maps = []
    for b in range(B):
        xT = np.ascontiguousarray(
            x[b].T.reshape(NCI, 128, T).transpose(1, 0, 2)
        ).astype(bf)
        in_maps.append({
            "xT": xT, "wq": wq_p, "wk": wk_p, "wv": wv_p, "wp": wp_p,
            "biasb": biasb,
        })
    return in_maps


_CACHE = {}


def kernel(x, Wq, Wk, Wv, Wp, bp):
    from concourse.bass_utils import run_bass_kernel_spmd

    if "nc" not in _CACHE:
        _CACHE["nc"] = build_kernel()
    nc = _CACHE["nc"]
    in_maps = _prep_inputs(x, Wq, Wk, Wv, Wp, bp)
    res = run_bass_kernel_spmd(nc, in_maps, list(range(NCORES)))
    out = np.stack([res.results[b]["y"] for b in range(B)], axis=0)
    return out.astype(np.float32)


# revision 27
# speedup vs baseline: 1.0226x; 1.0029x over previous
"""Multi-head causal attention (B=8, T=2048, C=384, H=6, Dh=64) on 8 TRN2 cores.

Sharding: data-parallel over batch — core b computes batch element b end to end
(no collectives).

v2 design notes (vs v1):
  - Head-PAIR processing: heads (2bi, 2bi+1) live on partition halves
    [0:64) / [64:128) of hd-block bi. Their S matmuls (K=64) are issued
    back-to-back so the PE runs them CONCURRENTLY via row tiling
    (tile_position auto-derived from base_partition 0 / 64).
  - One wide ACTIVATE (exp) per chunk-pair over S2 [128, 2, 512-d]
    (both heads at once) — halves ACT instruction overhead.
  - PV fringe-trimmed: matmul only cols [d:512] (P[:, :d] is never read
    → no memsets). Softmax denominator via V_aug ones-row (M=65).
  - Normalization: denominator rows broadcast across partitions on
    GpSimd (partition_broadcast), reciprocal on DVE, then one
    tensor_tensor multiply straight out of PSUM O → attT (bf16).
    No PE transposes, no per-head STT chains.
  - Output projection packed at K=128 (3 matmuls per 128-token block),
    bias added by the PSUM→SBUF scalar_tensor_tensor copy.
  - QKV projections are NOT a separate phase: block j+1's projection
    trios are emitted interleaved between block j's attention pairs, so
    the (readiness+priority) Tile scheduler uses them as PE filler
    inside the latency-bound attention chunk chain — keeps the PE
    HAM-warm (cold-clock dropped ~128us -> ~32us).
  - PSUM budget (8 banks): S2 pair tiles 2x2, proj/Y 1, O pairs 3.
    Pool membership chosen so no attention-critical alloc waits on the
    outproj/proj rotation (Y lives in the proj pool, not the O pool).
"""

import numpy as np
import ml_dtypes

import concourse.bass as bass
import concourse.tile as tile
from concourse import bacc, mybir
from concourse.bass import ts, ds

F32 = mybir.dt.float32
BF16 = mybir.dt.bfloat16
AF = mybir.ActivationFunctionType
ALU = mybir.AluOpType

B, T, C = 8, 2048, 384
H, DH = 6, 64
SCALE = DH ** -0.5
NCORES = 8
TJ = 512            # q-block width
NJ = T // TJ        # 4 q-blocks
SC = 128            # s-chunk
NCI = C // 128      # 3 channel chunks
NCH = TJ // SC      # fringe chunks per q-block (4)


def build_kernel():
    nc = bacc.Bacc("TRN2", target_bir_lowering=False, debug=False)

    xT_d = nc.dram_tensor("xT", [128, NCI, T], BF16, kind="ExternalInput").ap()
    wq_d = nc.dram_tensor("wq", [128, NCI, C], BF16, kind="ExternalInput").ap()
    wk_d = nc.dram_tensor("wk", [128, NCI, C], BF16, kind="ExternalInput").ap()
    wv_d = nc.dram_tensor("wv", [128, NCI, C], BF16, kind="ExternalInput").ap()
    wp_d = nc.dram_tensor("wp", [128, NCI, C], BF16, kind="ExternalInput").ap()
    biasb_d = nc.dram_tensor("biasb", [128, 384], F32, kind="ExternalInput").ap()
    y_d = nc.dram_tensor("y", [T, C], F32, kind="ExternalOutput").ap()

    with tile.TileContext(nc) as tc:
        with tc.tile_pool(name="const", bufs=1) as cpool, \
             tc.tile_pool(name="s2p", bufs=2, space="PSUM") as s2p, \
             tc.tile_pool(name="projp", bufs=1, space="PSUM") as projp, \
             tc.tile_pool(name="op", bufs=3, space="PSUM") as op, \
             tc.tile_pool(name="p2p", bufs=8) as p2p, \
             tc.tile_pool(name="r2p", bufs=3) as r2p, \
             tc.tile_pool(name="ysbp", bufs=3) as ysbp:
            xT = cpool.tile([128, NCI, T], BF16)
            wq = cpool.tile([128, NCI, C], BF16)
            wk = cpool.tile([128, NCI, C], BF16)
            wv = cpool.tile([128, NCI, C], BF16)
            wp = cpool.tile([128, NCI, C], BF16)
            biasb = cpool.tile([128, 384], F32)
            QT = cpool.tile([128, NCI, T], BF16)
            KT = cpool.tile([128, NCI, T], BF16)
            attT = cpool.tile([128, NCI, T], BF16)
            Vt = cpool.tile([128, 16, H, 65], BF16)

            # DMA order matches first consumption: KT/QT trios need wk/wq
            # and xT blocks 0-1; wp/biasb are only read by the outproj
            nc.sync.dma_start(wk[:], wk_d[:])
            nc.sync.dma_start(wq[:], wq_d[:])
            for tcn in (0, 1):
                for ci in range(NCI):
                    nc.sync.dma_start(xT[:, ci, ts(tcn, TJ)],
                                      xT_d[:, ci, ts(tcn, TJ)])
            nc.sync.dma_start(wv[:], wv_d[:])
            for tcn in (2, 3):
                for ci in range(NCI):
                    nc.sync.dma_start(xT[:, ci, ts(tcn, TJ)],
                                      xT_d[:, ci, ts(tcn, TJ)])
            nc.sync.dma_start(wp[:], wp_d[:])
            nc.sync.dma_start(biasb[:], biasb_d[:])
            # whole-tile memset (contiguous); V copies below overwrite
            # cols 0:64 of each [h, 65] group, leaving col 64 == 1.0
            nc.gpsimd.memset(Vt[:], 1.0)

            def qk_trio(dst, w, tcn, pi, pool_op=False):
                def run():
                    pool = op if pool_op else projp
                    tag = "O" if pool_op else "proj"
                    ps = pool.tile([128, TJ], F32, tag=tag, name="ps")
                    for ci in range(NCI):
                        nc.tensor.matmul(
                            ps[:],
                            lhsT=w[:, ci, ts(pi, 128)],
                            rhs=xT[:, ci, ts(tcn, TJ)],
                            start=(ci == 0), stop=(ci == NCI - 1),
                        )
                    nc.vector.tensor_copy(dst[:, pi, ts(tcn, TJ)], ps[:])
                return run

            def v_trio(si, pool_op=False):
                def run():
                    pool = op if pool_op else projp
                    tag = "O" if pool_op else "proj"
                    ps = pool.tile([128, TJ], F32, tag=tag, name="psv")
                    for ci in range(NCI):
                        nc.tensor.matmul(
                            ps[:, 0:C],
                            lhsT=xT[:, ci, ts(si, 128)],
                            rhs=wv[:, ci, :],
                            start=(ci == 0), stop=(ci == NCI - 1),
                        )
                    nc.vector.tensor_copy(
                        Vt[:, si, :, 0:64],
                        ps[:, 0:C].rearrange("p (h d) -> p h d", h=H),
                    )
                return run

            def emit_outproj(j, spread=False):
                # output projection for q-block j
                for q in range(NCH):
                    tb = NCH * j + q
                    if spread and q % 2:
                        Y = op.tile([128, TJ], F32, tag="O", name="Y")[:, 0:C]
                    else:
                        Y = projp.tile([128, TJ], F32, tag="proj", name="Y")[:, 0:C]
                    for bi in range(NCI):
                        nc.tensor.matmul(
                            Y[:],
                            lhsT=attT[:, bi, ts(tb, 128)],
                            rhs=wp[:, bi, :],
                            start=(bi == 0), stop=(bi == NCI - 1),
                        )
                    Ysb = ysbp.tile([128, C], F32, tag="Ysb")
                    nc.vector.scalar_tensor_tensor(
                        out=Ysb[:], in0=Y[:], scalar=1.0, in1=biasb[:],
                        op0=ALU.mult, op1=ALU.add,
                    )
                    nc.sync.dma_start(y_d[ts(tb, 128), :], Ysb[:])

            # Process q-blocks in order [1, 2, 3, 0]: long blocks first so
            # every j-boundary pipeline drain is absorbed by following
            # work, and tiny j0 covers j3's drain at the end.
            JORDER = [1, 2, 3, 0]
            # upfront: everything attention(j1) needs — KT cols 0:1024
            # (tcn 0+1), QT block 1, V chunks 0..7 — ordered so pair bi=0
            # becomes ready first; spread across both PSUM pools
            upfront = []
            for pi in range(NCI):
                upfront += [qk_trio(KT, wk, 0, pi, pool_op=False),
                            qk_trio(KT, wk, 1, pi, pool_op=True),
                            qk_trio(QT, wq, 1, pi, pool_op=False)]
            upfront += [v_trio(si, pool_op=(si % 2 == 1)) for si in range(8)]
            # filler trios for each later block
            TRIOS = {
                2: [qk_trio(KT, wk, 2, pi) for pi in range(NCI)]
                   + [qk_trio(QT, wq, 2, pi) for pi in range(NCI)]
                   + [v_trio(si) for si in range(8, 12)],
                3: [qk_trio(KT, wk, 3, pi) for pi in range(NCI)]
                   + [qk_trio(QT, wq, 3, pi) for pi in range(NCI)]
                   + [v_trio(si) for si in range(12, 16)],
                0: [qk_trio(QT, wq, 0, pi) for pi in range(NCI)],
            }
            for t in upfront:
                t()
            for jx, j in enumerate(JORDER):
                # next block's proj trios get interleaved between this
                # block's attention pairs (PE filler in the static order)
                pending = TRIOS.get(JORDER[jx + 1], []) \
                    if jx + 1 < len(JORDER) else []

                # ---- attention for q-block j, head pairs ----
                nch = NCH * j + NCH  # s-chunks for this q-block
                for bi in range(NCI):
                    h0, h1 = 2 * bi, 2 * bi + 1
                    O0 = op.tile([65, TJ], F32, tag="O")
                    O1 = op.tile([65, TJ], F32, tag="O")
                    for i in range(nch):
                        fringe = i >= NCH * j
                        d = SC * i - TJ * j if fringe else 0
                        S2 = s2p.tile([128, 2, TJ], F32, tag="S2")
                        # the two heads' S matmuls target different PE row
                        # groups (K=64 at partitions 0/64) → run concurrently
                        nc.tensor.matmul(
                            S2[:, 0, d:TJ],
                            lhsT=KT[0:64, bi, ts(i, SC)],
                            rhs=QT[0:64, bi, ds(j * TJ + d, TJ - d)],
                            start=True, stop=True,
                        )
                        nc.tensor.matmul(
                            S2[:, 1, d:TJ],
                            lhsT=KT[64:128, bi, ts(i, SC)],
                            rhs=QT[64:128, bi, ds(j * TJ + d, TJ - d)],
                            start=True, stop=True,
                        )
                        P2 = p2p.tile([128, 2, TJ], BF16, tag="P2")
                        nc.scalar.activation(P2[:, :, d:TJ], S2[:, :, d:TJ],
                                             AF.Exp, scale=SCALE)
                        if fringe:
                            # diagonal window [d, d+128): keep iff p <= f
                            for half in range(2):
                                nc.gpsimd.affine_select(
                                    out=P2[:, half, d:d + SC],
                                    in_=P2[:, half, d:d + SC],
                                    pattern=[[1, SC]],
                                    compare_op=ALU.is_ge,
                                    fill=0.0, base=0, channel_multiplier=-1,
                                )
                        nc.tensor.matmul(
                            O0[:, d:TJ],
                            lhsT=Vt[:, i, h0, :],
                            rhs=P2[:, 0, d:TJ],
                            start=(i == 0), stop=(i == nch - 1),
                        )
                        nc.tensor.matmul(
                            O1[:, d:TJ],
                            lhsT=Vt[:, i, h1, :],
                            rhs=P2[:, 1, d:TJ],
                            start=(i == 0), stop=(i == nch - 1),
                        )
    # normalization: reciprocal of the denom rows straight out of
                    # PSUM (approx_fast: ~18 bits, denominators are >= 1),
                    # broadcast over partitions 0:64 on gpsimd, multiply.
                    # All DVE input operands stay at partition base 0
                    # (mismatched in0/in1 bases read wrong data); only
                    # outputs are partition-shifted.
                    dA = r2p.tile([1, TJ], F32, tag="dA")
                    dB = r2p.tile([1, TJ], F32, tag="dB")
                    rA = r2p.tile([1, TJ], F32, tag="rA")
                    rB = r2p.tile([1, TJ], F32, tag="rB")
                    nc.vector.tensor_copy(dA[:], O0[64:65, :])
                    nc.vector.tensor_copy(dB[:], O1[64:65, :])
                    nc.vector.reciprocal_approx_fast(rA[:], dA[:])
                    nc.vector.reciprocal_approx_fast(rB[:], dB[:])
                    RA = r2p.tile([64, TJ], F32, tag="RA")
                    RB = r2p.tile([64, TJ], F32, tag="RB")
                    nc.gpsimd.partition_broadcast(RA[:], rA[:])
                    nc.gpsimd.partition_broadcast(RB[:], rB[:])
                    nc.vector.tensor_tensor(
                        out=attT[0:64, bi, ts(j, TJ)], in0=O0[0:64, :],
                        in1=RA[:], op=ALU.mult,
                    )
                    nc.vector.tensor_tensor(
                        out=attT[64:128, bi, ts(j, TJ)], in0=O1[0:64, :],
                        in1=RB[:], op=ALU.mult,
                    )
                    # interleave next-block proj trios between pairs,
                    # back-loaded to cover the j-boundary pipeline drain
                    nshare = (2, 2, 10)[bi] if pending else 0
                    for _ in range(min(nshare, len(pending))):
                        pending.pop(0)()

                emit_outproj(j, spread=(jx == len(JORDER) - 1))

    nc.compile()
    return nc


def _prep_inputs(x, Wq, Wk, Wv, Wp, bp):
    """Host-side shard + layout prep. Returns per-core input maps."""
    bf = ml_dtypes.bfloat16
    x = np.asarray(x, dtype=np.float32)

    def pack_w(W):  # [H, C, Dh] -> [128, NCI, H*Dh]
        Whd = np.transpose(np.asarray(W, np.float32), (1, 0, 2)).reshape(C, H * DH)
        return np.ascontiguousarray(
            Whd.reshape(NCI, 128, H * DH).transpose(1, 0, 2)
        ).astype(bf)

    wq_p, wk_p, wv_p = pack_w(Wq), pack_w(Wk), pack_w(Wv)
    wp_p = np.ascontiguousarray(
        np.asarray(Wp, np.float32).reshape(NCI, 128, C).transpose(1, 0, 2)
    ).astype(bf)

    biasb = np.broadcast_to(np.asarray(bp, np.float32), (128, C)).copy()

    in_maps = []
    for b in range(B):
        xT = np.ascontiguousarray(
            x[b].T.reshape(NCI, 128, T).transpose(1, 0, 2)
        ).astype(bf)
        in_maps.append({
            "xT": xT, "wq": wq_p, "wk": wk_p, "wv": wv_p, "wp": wp_p,
            "biasb": biasb,
        })
    return in_maps


_CACHE = {}


def kernel(x, Wq, Wk, Wv, Wp, bp):
    from concourse.bass_utils import run_bass_kernel_spmd

    if "nc" not in _CACHE:
        _CACHE["nc"] = build_kernel()
    nc = _CACHE["nc"]
    in_maps = _prep_inputs(x, Wq, Wk, Wv, Wp, bp)
    res = run_bass_kernel_spmd(nc, in_maps, list(range(NCORES)))
    out = np.stack([res.results[b]["y"] for b in range(B)], axis=0)
    return out.astype(np.float32)
